# revision 8
# baseline (speedup 1.0000x reference)
"""Trainium2 Bass kernel for ConvolutionalSelfAttention.

Math (per image, all exact reformulations of the reference):
  - bilinear downsample 128->16 (half-pixel, no antialias) == average of the
    2x2 pixel block at rows/cols {8i+3, 8i+4}   (verified vs jax.image.resize)
  - q/k/v projections + 256-token attention at 16x16
  - out-projection with Wo is fused with the per-channel multiplier
    mult = tanh(4*gamma+2.5)*0.5 (host-folded into Wo)
  - bilinear upsample 16->128 is a fixed sparse 128x16 matrix U per axis;
    both axis passes are done as TensorE matmuls with host-built block
    matrices so the final layout is [pixel-partition, channel-free] which
    DMAs out fully contiguously.

Distribution: pure data-parallel over batch, 2 images per NeuronCore x 8.
"""

import numpy as np
import ml_dtypes

import concourse.bass as bass
import concourse.bacc as bacc
import concourse.mybir as mybir
import concourse.tile as tile
from concourse.bass_utils import run_bass_kernel_spmd

F32 = mybir.dt.float32
BF16 = mybir.dt.bfloat16

B, H, W, C = 16, 128, 128, 256
CA = 128
RH = RW = 16
N_CORES = 8
BPC = B // N_CORES  # images per core

_CACHE = {}


# ----------------------------------------------------------------------------
# host-side constant matrices
# ----------------------------------------------------------------------------

def _upsample_matrix():
    """U[p, i]: 128-out bilinear upsample weights over 16 inputs (clamped)."""
    U = np.zeros((128, 16), np.float64)
    for p in range(128):
        src = (p - 3.5) / 8.0
        i0 = int(np.floor(src))
        w = src - i0
        U[p, min(max(i0, 0), 15)] += 1.0 - w
        U[p, min(max(i0 + 1, 0), 15)] += w
    return U.astype(np.float32)


def _host_consts():
    U = _upsample_matrix()

    # row-pass block matrices: zr[(p,j) chunk m] += R[kc][m].T @ z[chunk kc]
    # Token order within chunk kc is t*16+j for source row i = 2t+kc (the
    # even/odd-row interleave produced by the downsample stage), so
    # R[kc*16+m][t*16+j, (p-8m)*16+j] = U[p, 2t+kc]
    rmat = np.zeros((32, 128, 128), np.float32)
    jj = np.arange(16)
    for kc in range(2):
        for m in range(16):
            for t in range(8):
                for pp in range(8):
                    rmat[kc * 16 + m, t * 16 + jj, pp * 16 + jj] = U[8 * m + pp, 2 * t + kc]

    # col-pass block matrices: out[row p=8m+t] = E[t].T @ zr[chunk m]
    # E[t][t*16+j, q] = U[q, j]
    emat = np.zeros((8, 128, 128), np.float32)
    for t in range(8):
        for j in range(16):
            emat[t, t * 16 + j, :] = U[:, j]

    ident = np.eye(128, dtype=np.float32)
    return rmat, emat, ident


# ----------------------------------------------------------------------------
# bass program
# ----------------------------------------------------------------------------

def _build_nc():
    nc = bacc.Bacc("TRN2", target_bir_lowering=False, debug=False)

    x_d = nc.dram_tensor("x", [BPC, H, W, C], F32, kind="ExternalInput")
    wq_d = nc.dram_tensor("wq", [C, CA], F32, kind="ExternalInput")
    wk_d = nc.dram_tensor("wk", [C, CA], F32, kind="ExternalInput")
    wv_d = nc.dram_tensor("wv", [C, CA], F32, kind="ExternalInput")
    wo_d = nc.dram_tensor("wo", [CA, C], BF16, kind="ExternalInput")
    rm_d = nc.dram_tensor("rmat", [32, 128, 128], BF16, kind="ExternalInput")
    em_d = nc.dram_tensor("emat", [8, 128, 128], BF16, kind="ExternalInput")
    id_d = nc.dram_tensor("ident", [128, 128], F32, kind="ExternalInput")
    out_d = nc.dram_tensor("out", [BPC, H, W, C], F32, kind="ExternalOutput")

    x_ap = x_d.ap()
    out_ap = out_d.ap()

    copy_toggle = [0]

    with tile.TileContext(nc) as tc:
        with (
            tc.tile_pool(name="const", bufs=1) as cpool,
            tc.tile_pool(name="work", bufs=2) as wpool,
            tc.tile_pool(name="ostage", bufs=3) as opool,
            tc.tile_pool(name="psA", bufs=4, space="PSUM") as psA,
            tc.tile_pool(name="psR", bufs=2, space="PSUM") as psR,
            tc.tile_pool(name="psO", bufs=2, space="PSUM") as psO,
        ):
            def copy_op(out, in_):
                # alternate PSUM->SBUF copies across DVE and ACT
                if copy_toggle[0] % 2 == 0:
                    nc.vector.tensor_copy(out, in_)
                else:
                    nc.scalar.copy(out, in_)
                copy_toggle[0] += 1

            # ---- constants to SBUF ----
            wq_sb = cpool.tile([128, 2 * CA], F32, tag="wq")
            wk_sb = cpool.tile([128, 2 * CA], F32, tag="wk")
            wv_sb = cpool.tile([128, 2 * CA], F32, tag="wv")
            for w_sb, w_d in ((wq_sb, wq_d), (wk_sb, wk_d), (wv_sb, wv_d)):
                nc.sync.dma_start(
                    out=w_sb[:].rearrange("p (g d) -> p g d", g=2),
                    in_=w_d.ap().rearrange("(g p) d -> p g d", g=2),
                )
            wo_sb = cpool.tile([128, C], BF16, tag="wo")
            nc.sync.dma_start(out=wo_sb[:], in_=wo_d.ap())
            rm_sb = cpool.tile([128, 32 * 128], BF16, tag="rmat")
            nc.sync.dma_start(
                out=rm_sb[:].rearrange("p (n d) -> p n d", n=32),
                in_=rm_d.ap().transpose([1, 0, 2]),
            )
            em_sb = cpool.tile([128, 8 * 128], BF16, tag="emat")
            nc.sync.dma_start(
                out=em_sb[:].rearrange("p (n d) -> p n d", n=8),
                in_=em_d.ap().transpose([1, 0, 2]),
            )
            id_sb = cpool.tile([128, 128], F32, tag="ident")
            nc.sync.dma_start(out=id_sb[:], in_=id_d.ap())

            for b in range(BPC):
                # ---- gather the 32x32 needed pixels of x ------------------
                # xraw free layout: (i2, dr, dc*256+c); partition = t*16+j
                # pixel: row = 16t + 8*i2 + 3 + dr, col = 8j + 3 + dc
                # Each DMA writes a plain [128, 512] free-slice (keeps Tile's
                # dependency tracking exact).
                xraw = wpool.tile([128, 2048], F32, tag="xraw")
                for i2 in range(2):
                    for dr in range(2):
                        src = (
                            x_ap[b]
                            .rearrange("(t r) w c -> t r w c", r=16)[:, 8 * i2 + 3 + dr]
                            .rearrange("t (j e) c -> t j (e c)", e=8)[:, :, 768:1280]
                        )  # [8, 16, 512] = (t, j, dcc); (t,j) = partition
                        off = (i2 * 2 + dr) * 512
                        nc.gpsimd.dma_start(
                            out=xraw[:, off:off + 512], in_=src
                        )

                # ---- downsample-sum + transpose via TensorE ---------------
                # xsT[ch] = [c-half on partitions, 256 tokens]  (sum of 4 taps;
                # the 0.25 scale is folded into wq/wk/wv on the host)
                xraw5 = xraw[:].rearrange(
                    "p (g dr dc c) -> p g dr dc c", g=2, dr=2, dc=2
                )
                xsT_sb = wpool.tile([128, 2 * 256], F32, tag="xsT")
                for ch in range(2):
                    ps = psA.tile([128, 256], F32, tag="ps")
                    for i2 in range(2):
                        # token column s = i2*128 + t*16 + j  <->  source row
                        # i = 2t+i2, col j (permuted token order; rmat absorbs
                        # it, softmax/attention are order-invariant)
                        k = 0
                        for dr in range(2):
                            for dc in range(2):
                                nc.tensor.matmul(
                                    ps[:, i2 * 128:(i2 + 1) * 128],
                                    xraw5[:, i2, dr, dc, ch * 128:(ch + 1) * 128],
                                    id_sb[:],
                                    start=(k == 0),
                                    stop=(k == 3),
                                )
                                k += 1
                    copy_op(xsT_sb[:].rearrange("p (g t) -> p g t", g=2)[:, ch], ps[:])

                xsT = xsT_sb[:].rearrange("p (g t) -> p g t", g=2)

                # ---- q/k/v projections ------------------------------------
                qT_sb = wpool.tile([128, 256], F32, tag="qT")
                kT_sb = wpool.tile([128, 256], F32, tag="kT")
                for dst_sb, w_sb in ((qT_sb, wq_sb), (kT_sb, wk_sb)):
                    ps = psA.tile([128, 256], F32, tag="ps")
                    for cc in range(2):
                        nc.tensor.matmul(
                            ps[:],
                            w_sb[:].rearrange("p (g d) -> p g d", g=2)[:, cc],
                            xsT[:, cc],
                            start=(cc == 0),
                            stop=(cc == 1),
                        )
                    copy_op(dst_sb[:], ps[:])

                v_sb = wpool.tile([128, 256], F32, tag="v")  # [tok-in-chunk, (kc, d)]
                ps_v = psA.tile([128, 256], F32, tag="ps")
                for tc_ in range(2):
                    for cc in range(2):
                        nc.tensor.matmul(
                            ps_v[:, tc_ * 128:(tc_ + 1) * 128],
                            xsT[:, cc, tc_ * 128:(tc_ + 1) * 128],
                            wv_sb[:].rearrange("p (g d) -> p g d", g=2)[:, cc],
                            start=(cc == 0),
                            stop=(cc == 1),
                        )
                copy_op(v_sb[:], ps_v[:])

                # ---- scores + softmax -------------------------------------
                sc_ps = psA.tile([128, 512], F32, tag="ps")
                for ncc in range(2):
                    nc.tensor.matmul(
                        sc_ps[:, ncc * 256:(ncc + 1) * 256],
                        qT_sb[:, ncc * 128:(ncc + 1) * 128],
                        kT_sb[:],
                        start=True,
                        stop=True,
                    )
                attn_sb = wpool.tile([128, 2 * 256], F32, tag="attn")
                stats = wpool.tile([128, 6], F32, tag="stats")
                for ncc in range(2):
                    scv = sc_ps[:, ncc * 256:(ncc + 1) * 256]
                    negmax = stats[:, ncc:ncc + 1]
                    sumexp = stats[:, 2 + ncc:3 + ncc]
                    rsum = stats[:, 4 + ncc:5 + ncc]
                    nc.vector.tensor_reduce(
                        negmax, scv, mybir.AxisListType.X, mybir.AluOpType.max,
                        negate=True,
                    )
                    nc.scalar.activation(
                        attn_sb[:].rearrange("p (g t) -> p g t", g=2)[:, ncc],
                        scv,
                        mybir.ActivationFunctionType.Exp,
                        bias=negmax,
                        accum_out=sumexp,
                    )
                    nc.vector.reciprocal(rsum, sumexp)
                    nc.vector.tensor_scalar_mul(
                        attn_sb[:].rearrange("p (g t) -> p g t", g=2)[:, ncc],
                        attn_sb[:].rearrange("p (g t) -> p g t", g=2)[:, ncc],
                        rsum,
                    )

                # ---- transpose attn (PE) ----------------------------------
                attnT_sb = wpool.tile([128, 2 * 256], F32, tag="attnT")
                attn3 = attn_sb[:].rearrange("p (g t) -> p g t", g=2)
                for kc in range(2):
                    psT = psA.tile([128, 256], F32, tag="ps")
                    for ncc in range(2):
                        nc.tensor.transpose(
                            psT[:, ncc * 128:(ncc + 1) * 128],
                            attn3[:, ncc, kc * 128:(kc + 1) * 128],
                            id_sb[:],
                        )
                    copy_op(attnT_sb[:].rearrange("p (g t) -> p g t", g=2)[:, kc], psT[:])

                # ---- oT = [d on partitions, 256 tokens] -------------------
                oT_ps = psA.tile([128, 256], F32, tag="ps")
                v3 = v_sb[:].rearrange("p (g d) -> p g d", g=2)
                attnT3 = attnT_sb[:].rearrange("p (g t) -> p g t", g=2)
                for kc in range(2):
                    nc.tensor.matmul(
                        oT_ps[:],
                        v3[:, kc],
                        attnT3[:, kc],
                        start=(kc == 0),
                        stop=(kc == 1),
                    )
                oTb_sb = wpool.tile([128, 256], BF16, tag="oTb")
                copy_op(oTb_sb[:], oT_ps[:])

                # ---- z = o @ (Wo * mult): [token chunks on partitions, c] -
                z_ps = psA.tile([128, 512], F32, tag="ps")
                for tc_ in range(2):
                    nc.tensor.matmul(
                        z_ps[:, tc_ * 256:(tc_ + 1) * 256],
                        oTb_sb[:, tc_ * 128:(tc_ + 1) * 128],
                        wo_sb[:],
                        start=True,
                        stop=True,
                    )
                zb_sb = wpool.tile([128, 2 * 256], BF16, tag="zb")
                copy_op(zb_sb[:], z_ps[:])
                zb3 = zb_sb[:].rearrange("p (g t) -> p g t", g=2)

                # ---- row-pass upsample (TensorE) --------------------------
                zrb_sb = wpool.tile([128, 16 * 256], BF16, tag="zrb")
                rm3 = rm_sb[:].rearrange("p (n d) -> p n d", n=32)
                zrb3 = zrb_sb[:].rearrange("p (n t) -> p n t", n=16)
                for m in range(16):
                    ps = psR.tile([128, 256], F32, tag="psr")
                    for kc in range(2):
                        nc.tensor.matmul(
                            ps[:],
                            rm3[:, kc * 16 + m],
                            zb3[:, kc],
                            start=(kc == 0),
                            stop=(kc == 1),
                        )
                    copy_op(zrb3[:, m], ps[:])

                # ---- col-pass upsample + store ----------------------------
                em3 = em_sb[:].rearrange("p (n d) -> p n d", n=8)
                ostage = None
                for m in range(16):
                    for t in range(0, 8, 2):
                        p = 8 * m + t
                        if p % 8 == 0:
                            ostage = opool.tile([128, 8 * 256], F32, tag="ostage")
                        pso = psO.tile([128, 512], F32, tag="pso")
                        for u in range(2):
                            nc.tensor.matmul(
                                pso[:, u * 256:(u + 1) * 256],
                                em3[:, t + u],
                                zrb3[:, m],
                                start=True,
                                stop=True,
                            )
                        off = ((p % 8) // 2) * 512
                        copy_op(ostage[:, off:off + 512], pso[:])
                        if p % 8 == 6:
                            p0 = (p // 8) * 8
                            nc.sync.dma_start(
                                out=out_ap[b, p0:p0 + 8].transpose([1, 0, 2]),
                                in_=ostage[:].rearrange("q (r c) -> q r c", r=8),
                            )

    nc.compile()
    return nc


def _get_nc():
    if "nc" not in _CACHE:
        _CACHE["nc"] = _build_nc()
    return _CACHE["nc"]


# ----------------------------------------------------------------------------
# host wrapper
# ----------------------------------------------------------------------------

def _make_in_maps(x, Wq, Wk, Wv, Wo, gamma_w):
    x = np.ascontiguousarray(np.asarray(x, np.float32))
    mult = np.tanh(4.0 * np.asarray(gamma_w, np.float32) + 2.5) * 0.5  # [C]
    wo_eff = (np.asarray(Wo, np.float32) * mult[None, :]).astype(ml_dtypes.bfloat16)
    rmat, emat, ident = _host_consts()
    common = {
        "wq": np.ascontiguousarray(np.asarray(Wq, np.float32) * 0.25),
        "wk": np.ascontiguousarray(np.asarray(Wk, np.float32) * 0.25),
        "wv": np.ascontiguousarray(np.asarray(Wv, np.float32) * 0.25),
        "wo": wo_eff,
        "rmat": rmat.astype(ml_dtypes.bfloat16),
        "emat": emat.astype(ml_dtypes.bfloat16),
        "ident": ident,
    }
    in_maps = []
    for i in range(N_CORES):
        m = dict(common)
        m["x"] = x[i * BPC:(i + 1) * BPC]
        in_maps.append(m)
    return in_maps


def run_spmd(inputs, **kwargs):
    nc = _get_nc()
    in_maps = _make_in_maps(**inputs)
    return run_bass_kernel_spmd(nc, in_maps, core_ids=list(range(N_CORES)), **kwargs)


def kernel(x, Wq, Wk, Wv, Wo, gamma_w):
    res = run_spmd(dict(x=x, Wq=Wq, Wk=Wk, Wv=Wv, Wo=Wo, gamma_w=gamma_w))
    outs = [res.results[i]["out"] for i in range(N_CORES)]
    return np.concatenate(outs, axis=0)
